# revision 44
# baseline (speedup 1.0000x reference)
"""Multi-head attention (B=2, L=2048, D=1024, H=16) on 8 trn2 NeuronCores.

Sharding: core c = (batch b = c // 4, head-group g = c % 4); each group owns 4
heads (256 dims). Q/K/V projections are column-parallel per group, attention is
fully local per (batch, head), fc is row-parallel with the 4 group partials of
each batch summed on the host.

Per-core dataflow (matmul operands bf16, PSUM accumulation fp32):
  qT,kT [256, L] = W @ x.T          (host supplies x.T and W.T slices)
  v     [L, 256] (+ ones column)    (bias via augmented contraction row)
  S.T   [k-block 128, q-span 512] = kT' qT   -> exp(S/8) on ScalarE -> p.T
  mask handled per 128x128 block: pure (nothing), dead (zeroed), mixed
  (multiply by 0/1 mask chunk) -- classification from the actual mask.
  u.T   [65, q-span] += v_aug.T p.T (row 64 = softmax denominators)
  ctx.T = u.T * broadcast(1/denom)  (broadcast via PE ones-column matmul)
  y     [L, 1024] = ctx.T' fcT (+ fc bias via augmented row, group 0 only)
"""

import numpy as np
import ml_dtypes

import concourse.bass as bass
import concourse.mybir as mybir
import concourse.tile as tile
from concourse import bacc, bass_utils

L = 2048
D = 1024
DK = 64
GH = 4            # heads per core
DG = 256          # dims per core
NB = L // 128     # 16 key/query blocks
NSPAN = L // 512  # 4 query spans
F32 = mybir.dt.float32
BF = mybir.dt.bfloat16
U8 = mybir.dt.uint8

_CACHE: dict = {}
LAST_EXEC_NS = None
TRACE = False


def _install_ntff_hook():
    """Register the axon NTFF profiling hook that this image's antenv lacks.

    Replicates trn_agent_boot.trn_boot._ntff_profile_via_ctypes against
    /opt/axon/libaxon_pjrt.so so run_bass_kernel_spmd(trace=True) works.
    """
    import contextlib
    import ctypes
    import sys
    import types

    try:
        from antenv.axon_hooks import get_axon_ntff_profile_hook  # noqa: F401
        return
    except ImportError:
        pass
    import antenv

    mod = types.ModuleType("antenv.axon_hooks")
    state = {"hook": None}
    mod.set_axon_ntff_profile_hook = lambda h: state.__setitem__("hook", h)
    mod.get_axon_ntff_profile_hook = lambda: state["hook"]
    sys.modules["antenv.axon_hooks"] = mod
    antenv.axon_hooks = mod

    so_path = "/opt/axon/libaxon_pjrt.so"
    lib = ctypes.CDLL(so_path)
    if not hasattr(lib, "axon_start_nrt_profile"):
        return
    lib.axon_start_nrt_profile.argtypes = [
        ctypes.POINTER(ctypes.c_int64),
        ctypes.c_size_t,
    ]
    lib.axon_start_nrt_profile.restype = ctypes.c_int64
    lib.axon_stop_nrt_profile.argtypes = [ctypes.c_char_p]
    lib.axon_stop_nrt_profile.restype = ctypes.c_int64

    @contextlib.contextmanager
    def _hook(output_dir, device_ids):
        import jax

        jax.devices()
        if device_ids:
            ids = (ctypes.c_int64 * len(device_ids))(*device_ids)
            rc = lib.axon_start_nrt_profile(ids, len(device_ids))
        else:
            rc = lib.axon_start_nrt_profile(None, 0)
        if rc != 0:
            raise RuntimeError(f"axon_start_nrt_profile rc={rc}")
        try:
            yield
        finally:
            n = lib.axon_stop_nrt_profile(str(output_dir).encode())
            print(f"profile: {n} file(s) written to {output_dir}", file=sys.stderr)

    state["hook"] = _hook


def _classify(mask2d: np.ndarray) -> np.ndarray:
    """cls[qb, kb]: 0 = all masked (dead), 1 = all unmasked (pure), 2 = mixed."""
    m = mask2d.astype(np.uint8).reshape(NB, 128, NB, 128)
    s = m.sum(axis=(1, 3))
    cls = np.full((NB, NB), 2, np.int8)
    cls[s == 0] = 0
    cls[s == 128 * 128] = 1
    return cls


def _mixed_list(cls):
    return [(qb, kb) for qb in range(NB) for kb in range(NB) if cls[qb, kb] == 2]


def _build(cls: np.ndarray, zv: bool = False, zf: bool = False):
    nc = bacc.Bacc("TRN2", target_bir_lowering=False, debug=False, num_devices=8)
    XTQ = nc.dram_tensor("XTQ", [D, L], BF, kind="ExternalInput").ap()
    XTK = nc.dram_tensor("XTK", [D, L], BF, kind="ExternalInput").ap()
    XTV = nc.dram_tensor("XTV", [D + 1, L], BF, kind="ExternalInput").ap()
    WQT = nc.dram_tensor("WQT", [D, DG], BF, kind="ExternalInput").ap()
    WKT = nc.dram_tensor("WKT", [D, DG], BF, kind="ExternalInput").ap()
    WVT = nc.dram_tensor("WVT", [D + 1, DG], BF, kind="ExternalInput").ap()
    BQ = nc.dram_tensor("BQ", [DG, 1], F32, kind="ExternalInput").ap()
    BK = nc.dram_tensor("BK", [DG, 1], F32, kind="ExternalInput").ap()
    FCT = nc.dram_tensor("FCT", [DG + 1, D], BF, kind="ExternalInput").ap()
    mixed = _mixed_list(cls)
    nmix = max(1, len(mixed))
    MCHUNKS = nc.dram_tensor("MCHUNKS", [nmix, 128, 128], U8, kind="ExternalInput").ap()
    Y = nc.dram_tensor("Y", [L, D], F32, kind="ExternalOutput").ap()

    # per-span live key blocks (shared by all heads; mask broadcasts)
    span_kbs = []
    for s in range(NSPAN):
        kbs = [kb for kb in range(NB) if any(cls[4 * s + j, kb] for j in range(4))]
        assert kbs, f"query span {s} has no unmasked keys"
        span_kbs.append(kbs)

    Exp = mybir.ActivationFunctionType.Exp

    with tile.TileContext(nc) as tc:
        with (
            tc.tile_pool(name="w", bufs=1) as wp,
            tc.tile_pool(name="xs", bufs=4) as xp,
            tc.tile_pool(name="keep", bufs=1) as kp,
            tc.tile_pool(name="ptp", bufs=8) as ptp,
            tc.tile_pool(name="sm", bufs=3) as smp,
            tc.tile_pool(name="ev", bufs=2) as evp,
            tc.tile_pool(name="pout", bufs=2, space="PSUM") as pout,
            tc.tile_pool(name="psc", bufs=3, space="PSUM") as psc,
        ):
            # ---------------- persistent activations ----------------
            qt_sb = [kp.tile([128, L], BF, tag=f"qt{i}", name=f"qt{i}") for i in range(2)]
            kt_sb = [kp.tile([128, L], BF, tag=f"kt{i}", name=f"kt{i}") for i in range(2)]
            ctx_sb = [kp.tile([128, L], BF, tag=f"ctx{i}", name=f"ctx{i}") for i in range(2)]
            v_sb = kp.tile([128, NB, GH, 65], BF, tag="vsb")

            zeros = wp.tile([128, 64], F32, tag="zeros")
            nc.vector.memset(zeros[:], 0.0)
            zrow = wp.tile([1, L], F32, tag="zrow")
            nc.vector.memset(zrow[:], 0.0)
            ones64 = wp.tile([1, 64], BF, tag="ones64")
            nc.scalar.add(ones64[:], zeros[0:1, :], 1.0)
            ctx1 = wp.tile([1, L], BF, tag="ctx1")
            nc.scalar.add(ctx1[:], zrow[:], 1.0)
            nc.scalar.add(
                v_sb[:, :, :, 64:65],
                zeros[:].rearrange("p (a b c) -> p a b c", a=NB, b=GH),
                1.0,
            )

            # ---------------- weights (one batched DMA per tensor) ---------
            wqt = wp.tile([128, 8, DG], BF, tag="wqt")
            wkt = wp.tile([128, 8, DG], BF, tag="wkt")
            wvt = wp.tile([128, 8, DG], BF, tag="wvt")
            vrow = wp.tile([1, DG], BF, tag="vrow")
            fct = wp.tile([128, 2, D], BF, tag="fct")
            fcb = wp.tile([1, D], BF, tag="fcb")
            bq = wp.tile([128, 2, 1], F32, tag="bq")
            bk = wp.tile([128, 2, 1], F32, tag="bk")
            for kt in range(8):
                nc.sync.dma_start(out=wqt[:, kt], in_=WQT[kt * 128:(kt + 1) * 128])
            nc.sync.dma_start(out=bq[:], in_=BQ.rearrange("(i p) o -> p i o", p=128))

            def load_half(src, half, name):
                # one DMA per k-tile so the 8-matmul accumulation chain can
                # start as soon as its first operand lands
                t = xp.tile([128, 8, 1024], BF, tag="xt", name=name)
                for kt in range(8):
                    nc.sync.dma_start(
                        out=t[:, kt],
                        in_=src[kt * 128:(kt + 1) * 128,
                                half * 1024:(half + 1) * 1024],
                    )
                return t

            def proj_qk_half(xt, half, wt, bias, dst):
                for mch in range(2):
                    for sl in range(2):
                        s = half * 2 + sl
                        p = pout.tile([128, 512], F32, tag="out", name="pqk")
                        for kt in range(8):
                            nc.tensor.matmul(
                                p[:],
                                wt[:, kt, mch * 128:(mch + 1) * 128],
                                xt[:, kt, sl * 512:(sl + 1) * 512],
                                start=(kt == 0),
                                stop=(kt == 7),
                            )
                        nc.vector.tensor_scalar_add(
                            dst[mch][:, s * 512:(s + 1) * 512], p[:], bias[:, mch]
                        )

            t = load_half(XTQ, 0, "xtq")
            nc.sync.dma_start(out=wkt[:], in_=WKT.rearrange("(kt p) m -> p kt m", p=128))
            nc.sync.dma_start(out=bk[:], in_=BK.rearrange("(i p) o -> p i o", p=128))
            proj_qk_half(t, 0, wqt, bq, qt_sb)
            t = load_half(XTQ, 1, "xtq")
            nc.sync.dma_start(out=wvt[:], in_=WVT[0:D].rearrange("(kt p) m -> p kt m", p=128))
            nc.sync.dma_start(out=vrow[:], in_=WVT[D:D + 1])
            proj_qk_half(t, 1, wqt, bq, qt_sb)
            for half in range(2):
                t = load_half(XTK, half, "xtk")
                proj_qk_half(t, half, wkt, bk, kt_sb)

            # ---------------- v projection ----------------
            xr = xp.tile([1, L], BF, tag="xtr", bufs=1, name="xr")
            nc.sync.dma_start(out=xr[:], in_=XTV[D:D + 1])
            nc.sync.dma_start(out=fct[:], in_=FCT[0:DG].rearrange("(i p) m -> p i m", p=128))
            nc.sync.dma_start(out=fcb[:], in_=FCT[DG:DG + 1])

            for half in range(2):
                xtv = load_half(XTV, half, "xtv")
                for lbl in range(8):
                    lb = half * 8 + lbl
                    p = pout.tile([128, DG], F32, tag="out", name="pv")
                    for kt in range(8):
                        nc.tensor.matmul(
                            p[:],
                            xtv[:, kt, lbl * 128:(lbl + 1) * 128],
                            wvt[:, kt],
                            start=(kt == 0),
                            stop=(zv and kt == 7),
                        )
                    if not zv:
                        nc.tensor.matmul(
                            p[:], xr[:, lb * 128:(lb + 1) * 128], vrow[:],
                            start=False, stop=True,
                        )
                    nc.vector.tensor_copy(
                        v_sb[:, lb, :, 0:64], p[:].rearrange("p (h d) -> p h d", h=GH)
                    )

            # 0/1 chunks for mixed mask blocks, one DMA + one convert
            m01_idx = {qk: i for i, qk in enumerate(mixed)}
            m01_all = wp.tile([128, nmix, 128], BF, tag="m01")
            if mixed:
                mstage = wp.tile([128, nmix, 128], U8, tag="mstage")
                nc.sync.dma_start(out=mstage[:], in_=MCHUNKS.rearrange("n p c -> p n c"))
                nc.scalar.copy(m01_all[:], mstage[:])

            # ---------------- attention ----------------
            def first_col(kb, s):
                if kb == span_kbs[s][0]:
                    return 0
                j0 = next(j for j in range(4) if cls[4 * s + j, kb])
                return j0 * 128

            def normalize(hi, ho, s, outp):
                srow = smp.tile([1, 512], F32, tag="srow", name="srow")
                nc.vector.tensor_copy(srow[:], outp[64:65, :])
                rc32 = smp.tile([1, 512], F32, tag="rc32", name="rc32")
                nc.vector.reciprocal_approx_fast(rc32[:], srow[:])
                rcb = smp.tile([1, 512], BF, tag="rcb", name="rcb")
                nc.vector.tensor_copy(rcb[:], rc32[:])
                bcp = psc.tile([64, 512], F32, tag="sc", name="bcp")
                nc.tensor.matmul(bcp[:], ones64[:], rcb[:], start=True, stop=True)
                bcs = evp.tile([64, 512], BF, tag="bcs", name="bcs")
                nc.vector.tensor_copy(bcs[:], bcp[:])
                nc.vector.tensor_mul(
                    ctx_sb[hi][ho:ho + 64, s * 512:(s + 1) * 512],
                    outp[0:64, :],
                    bcs[:],
                )

            pending = None
            for h in range(GH):
                hi, ho = h // 2, (h % 2) * 64
                for s in range(NSPAN):
                    kbs = span_kbs[s]
                    outp = pout.tile([65, 512], F32, tag="out", name=f"out{h}_{s}")
                    ngrp = (len(kbs) + 1) // 2
                    for gi in range(ngrp):
                        grp = kbs[gi * 2:gi * 2 + 2]
                        scp = psc.tile([128, 1024], F32, tag="sc", name="scp")
                        pt = ptp.tile([128, 1024], BF, tag="pt", name="pt")
                        c0s = [first_col(kb, s) for kb in grp]
                        for idx, kb in enumerate(grp):
                            off = idx * 512
                            nc.tensor.matmul(
                                scp[:, off + c0s[idx]:off + 512],
                                kt_sb[hi][ho:ho + 64, kb * 128:(kb + 1) * 128],
                                qt_sb[hi][ho:ho + 64, s * 512 + c0s[idx]:(s + 1) * 512],
                                start=True,
                                stop=True,
                            )
                        lo = c0s[0]
                        end = (len(grp) - 1) * 512 + 512
                        nc.scalar.activation(pt[:, lo:end], scp[:, lo:end], Exp, scale=0.125)
                        for idx, kb in enumerate(grp):
                            off = idx * 512
                            for j in range(c0s[idx] // 128, 4):
                                qb = 4 * s + j
                                c = cls[qb, kb]
                                if c == 0:
                                    nc.vector.memset(pt[:, off + j * 128:off + (j + 1) * 128], 0.0)
                                elif c == 2:
                                    nc.vector.tensor_mul(
                                        pt[:, off + j * 128:off + (j + 1) * 128],
                                        pt[:, off + j * 128:off + (j + 1) * 128],
                                        m01_all[:, m01_idx[(qb, kb)], :],
                                    )
                            nc.tensor.matmul(
                                outp[:, c0s[idx]:],
                                v_sb[:, kb, h, :],
                                pt[:, off + c0s[idx]:off + 512],
                                start=(kb == kbs[0]),
                                stop=(kb == kbs[-1]),
                                skip_group_check=True,
                            )
                        # flush the previous span's normalize mid-stream so its
                        # reciprocal chain never stalls the PE
                        if gi == min(1, ngrp - 1) and pending is not None:
                            normalize(*pending)
                            pending = None
                    pending = (hi, ho, s, outp)
            if pending is not None:
                normalize(*pending)

            # ---------------- fc ----------------
            for lb in range(NB):
                ys = evp.tile([128, 1024], F32, tag="ys")
                for nh in range(2):
                    yp = pout.tile([128, 512], F32, tag="out", name="yp")
                    nc.tensor.matmul(
                        yp[:], ctx_sb[0][:, lb * 128:(lb + 1) * 128],
                        fct[:, 0, nh * 512:(nh + 1) * 512], start=True, stop=False,
                    )
                    nc.tensor.matmul(
                        yp[:], ctx_sb[1][:, lb * 128:(lb + 1) * 128],
                        fct[:, 1, nh * 512:(nh + 1) * 512], start=False, stop=zf,
                    )
                    if not zf:
                        nc.tensor.matmul(
                            yp[:], ctx1[:, lb * 128:(lb + 1) * 128],
                            fcb[:, nh * 512:(nh + 1) * 512], start=False, stop=True,
                        )
                    nc.vector.tensor_copy(ys[:, nh * 512:(nh + 1) * 512], yp[:])
                nc.sync.dma_start(out=Y[lb * 128:(lb + 1) * 128, :], in_=ys[:])

    nc.compile()
    return nc


def kernel(Q, K, V, mask, Wq_w, Wq_b, Wk_w, Wk_b, Wv_w, Wv_b, fc_w, fc_b):
    global LAST_EXEC_NS
    Q = np.asarray(Q, np.float32)
    K = np.asarray(K, np.float32)
    V = np.asarray(V, np.float32)
    mask2d = np.asarray(mask).reshape(L, L).astype(bool)
    Wq_w = np.asarray(Wq_w, np.float32)
    Wq_b = np.asarray(Wq_b, np.float32)
    Wk_w = np.asarray(Wk_w, np.float32)
    Wk_b = np.asarray(Wk_b, np.float32)
    Wv_w = np.asarray(Wv_w, np.float32)
    Wv_b = np.asarray(Wv_b, np.float32)
    fc_w = np.asarray(fc_w, np.float32)
    fc_b = np.asarray(fc_b, np.float32)

    cls = _classify(mask2d)
    zv = not Wv_b.any()
    zf = not fc_b.any()
    key = (cls.tobytes(), zv, zf)
    if key not in _CACHE:
        _CACHE[key] = _build(cls, zv, zf)
    nc = _CACHE[key]

    bf = ml_dtypes.bfloat16
    mixed = _mixed_list(cls)
    if mixed:
        mchunks = np.stack([
            np.ascontiguousarray(mask2d[qb * 128:(qb + 1) * 128, kb * 128:(kb + 1) * 128].T)
            for qb, kb in mixed
        ]).astype(np.uint8)
    else:
        mchunks = np.zeros((1, 128, 128), np.uint8)
    ones_row = np.ones((1, L), np.float32)

    xt = {}
    for b in range(2):
        xt[("Q", b)] = np.ascontiguousarray(Q[b].T).astype(bf)
        xt[("K", b)] = np.ascontiguousarray(K[b].T).astype(bf)
        xt[("V", b)] = np.concatenate([np.ascontiguousarray(V[b].T), ones_row], 0).astype(bf)

    in_maps = []
    for c in range(8):
        b, g = c // 4, c % 4
        sl = slice(g * DG, (g + 1) * DG)
        fc_last = fc_b[None, :] if g == 0 else np.zeros((1, D), np.float32)
        in_maps.append({
            "XTQ": xt[("Q", b)],
            "XTK": xt[("K", b)],
            "XTV": xt[("V", b)],
            "WQT": np.ascontiguousarray(Wq_w[sl, :].T).astype(bf),
            "WKT": np.ascontiguousarray(Wk_w[sl, :].T).astype(bf),
            "WVT": np.concatenate(
                [np.ascontiguousarray(Wv_w[sl, :].T), Wv_b[sl][None, :]], 0
            ).astype(bf),
            "BQ": np.ascontiguousarray(Wq_b[sl].reshape(DG, 1)),
            "BK": np.ascontiguousarray(Wk_b[sl].reshape(DG, 1)),
            "FCT": np.concatenate(
                [np.ascontiguousarray(fc_w[:, sl].T), fc_last], 0
            ).astype(bf),
            "MCHUNKS": mchunks,
        })

    if TRACE:
        _install_ntff_hook()
    res = bass_utils.run_bass_kernel_spmd(
        nc, in_maps, core_ids=list(range(8)),
        trace=TRACE, trace_cores=list(range(8)) if TRACE else None,
    )
    LAST_EXEC_NS = res.exec_time_ns

    out = np.zeros((2, L, D), np.float32)
    for c in range(8):
        out[c // 4] += res.results[c]["Y"]
    return out


# revision 45
# speedup vs baseline: 1.2081x; 1.2081x over previous
"""Multi-head attention (B=2, L=2048, D=1024, H=16) on 8 trn2 NeuronCores.

Sharding: core c = (batch b = c // 4, head-group g = c % 4); each group owns 4
heads (256 dims). Q/K/V projections are column-parallel per group, attention is
fully local per (batch, head), fc is row-parallel with the 4 group partials of
each batch summed on the host.

Per-core dataflow (matmul operands bf16, PSUM accumulation fp32):
  qT,kT [256, L] = W @ x.T          (host supplies x.T and W.T slices)
  v     [L, 256] (+ ones column)    (bias via augmented contraction row)
  S.T   [k-block 128, q-span 512] = kT' qT   -> exp(S/8) on ScalarE -> p.T
  mask handled per 128x128 block: pure (nothing), dead (zeroed), mixed
  (multiply by 0/1 mask chunk) -- classification from the actual mask.
  u.T   [65, q-span] += v_aug.T p.T (row 64 = softmax denominators)
  ctx.T = u.T * broadcast(1/denom)  (broadcast via PE ones-column matmul)
  y     [L, 1024] = ctx.T' fcT (+ fc bias via augmented row, group 0 only)
"""

import numpy as np
import ml_dtypes

import concourse.bass as bass
import concourse.mybir as mybir
import concourse.tile as tile
from concourse import bacc, bass_utils

L = 2048
D = 1024
DK = 64
GH = 4            # heads per core
DG = 256          # dims per core
NB = L // 128     # 16 key/query blocks
NSPAN = L // 512  # 4 query spans
F32 = mybir.dt.float32
BF = mybir.dt.bfloat16
U8 = mybir.dt.uint8

_CACHE: dict = {}
LAST_EXEC_NS = None
TRACE = False


def _install_ntff_hook():
    """Register the axon NTFF profiling hook that this image's antenv lacks.

    Replicates trn_agent_boot.trn_boot._ntff_profile_via_ctypes against
    /opt/axon/libaxon_pjrt.so so run_bass_kernel_spmd(trace=True) works.
    """
    import contextlib
    import ctypes
    import sys
    import types

    try:
        from antenv.axon_hooks import get_axon_ntff_profile_hook  # noqa: F401
        return
    except ImportError:
        pass
    import antenv

    mod = types.ModuleType("antenv.axon_hooks")
    state = {"hook": None}
    mod.set_axon_ntff_profile_hook = lambda h: state.__setitem__("hook", h)
    mod.get_axon_ntff_profile_hook = lambda: state["hook"]
    sys.modules["antenv.axon_hooks"] = mod
    antenv.axon_hooks = mod

    so_path = "/opt/axon/libaxon_pjrt.so"
    lib = ctypes.CDLL(so_path)
    if not hasattr(lib, "axon_start_nrt_profile"):
        return
    lib.axon_start_nrt_profile.argtypes = [
        ctypes.POINTER(ctypes.c_int64),
        ctypes.c_size_t,
    ]
    lib.axon_start_nrt_profile.restype = ctypes.c_int64
    lib.axon_stop_nrt_profile.argtypes = [ctypes.c_char_p]
    lib.axon_stop_nrt_profile.restype = ctypes.c_int64

    @contextlib.contextmanager
    def _hook(output_dir, device_ids):
        import jax

        jax.devices()
        if device_ids:
            ids = (ctypes.c_int64 * len(device_ids))(*device_ids)
            rc = lib.axon_start_nrt_profile(ids, len(device_ids))
        else:
            rc = lib.axon_start_nrt_profile(None, 0)
        if rc != 0:
            raise RuntimeError(f"axon_start_nrt_profile rc={rc}")
        try:
            yield
        finally:
            n = lib.axon_stop_nrt_profile(str(output_dir).encode())
            print(f"profile: {n} file(s) written to {output_dir}", file=sys.stderr)

    state["hook"] = _hook


def _classify(mask2d: np.ndarray) -> np.ndarray:
    """cls[qb, kb]: 0 = all masked (dead), 1 = all unmasked (pure), 2 = mixed."""
    m = mask2d.astype(np.uint8).reshape(NB, 128, NB, 128)
    s = m.sum(axis=(1, 3))
    cls = np.full((NB, NB), 2, np.int8)
    cls[s == 0] = 0
    cls[s == 128 * 128] = 1
    return cls


def _mixed_list(cls):
    return [(qb, kb) for qb in range(NB) for kb in range(NB) if cls[qb, kb] == 2]


def _build(cls: np.ndarray, zv: bool = False, zf: bool = False):
    nc = bacc.Bacc("TRN2", target_bir_lowering=False, debug=False, num_devices=8)
    XTQ = nc.dram_tensor("XTQ", [D, L], BF, kind="ExternalInput").ap()
    XTK = nc.dram_tensor("XTK", [D, L], BF, kind="ExternalInput").ap()
    XTV = nc.dram_tensor("XTV", [D + 1, L], BF, kind="ExternalInput").ap()
    WQT = nc.dram_tensor("WQT", [D, DG], BF, kind="ExternalInput").ap()
    WKT = nc.dram_tensor("WKT", [D, DG], BF, kind="ExternalInput").ap()
    WVT = nc.dram_tensor("WVT", [D + 1, DG], BF, kind="ExternalInput").ap()
    BQ = nc.dram_tensor("BQ", [DG, 1], F32, kind="ExternalInput").ap()
    BK = nc.dram_tensor("BK", [DG, 1], F32, kind="ExternalInput").ap()
    FCT = nc.dram_tensor("FCT", [DG + 1, D], BF, kind="ExternalInput").ap()
    mixed = _mixed_list(cls)
    nmix = max(1, len(mixed))
    MCHUNKS = nc.dram_tensor("MCHUNKS", [nmix, 128, 128], U8, kind="ExternalInput").ap()
    Y = nc.dram_tensor("Y", [L, D], F32, kind="ExternalOutput").ap()

    # per-span live key blocks (shared by all heads; mask broadcasts)
    span_kbs = []
    for s in range(NSPAN):
        kbs = [kb for kb in range(NB) if any(cls[4 * s + j, kb] for j in range(4))]
        assert kbs, f"query span {s} has no unmasked keys"
        span_kbs.append(kbs)

    Exp = mybir.ActivationFunctionType.Exp

    with tile.TileContext(nc) as tc:
        with (
            tc.tile_pool(name="w", bufs=1) as wp,
            tc.tile_pool(name="xs", bufs=4) as xp,
            tc.tile_pool(name="keep", bufs=1) as kp,
            tc.tile_pool(name="ptp", bufs=8) as ptp,
            tc.tile_pool(name="sm", bufs=3) as smp,
            tc.tile_pool(name="ev", bufs=2) as evp,
            tc.tile_pool(name="pout", bufs=2, space="PSUM") as pout,
            tc.tile_pool(name="psc", bufs=3, space="PSUM") as psc,
        ):
            # ---------------- persistent activations ----------------
            qt_sb = [kp.tile([128, L], BF, tag=f"qt{i}", name=f"qt{i}") for i in range(2)]
            kt_sb = [kp.tile([128, L], BF, tag=f"kt{i}", name=f"kt{i}") for i in range(2)]
            ctx_sb = [kp.tile([128, L], BF, tag=f"ctx{i}", name=f"ctx{i}") for i in range(2)]
            v_sb = kp.tile([128, NB, GH, 65], BF, tag="vsb")

            zeros = wp.tile([128, 64], F32, tag="zeros")
            nc.vector.memset(zeros[:], 0.0)
            zrow = wp.tile([1, L], F32, tag="zrow")
            nc.vector.memset(zrow[:], 0.0)
            ones64 = wp.tile([1, 64], BF, tag="ones64")
            nc.scalar.add(ones64[:], zeros[0:1, :], 1.0)
            ctx1 = wp.tile([1, L], BF, tag="ctx1")
            nc.scalar.add(ctx1[:], zrow[:], 1.0)
            nc.scalar.add(
                v_sb[:, :, :, 64:65],
                zeros[:].rearrange("p (a b c) -> p a b c", a=NB, b=GH),
                1.0,
            )

            # ---------------- weights (one batched DMA per tensor) ---------
            wqt = wp.tile([128, 8, DG], BF, tag="wqt")
            wkt = wp.tile([128, 8, DG], BF, tag="wkt")
            wvt = wp.tile([128, 8, DG], BF, tag="wvt")
            vrow = wp.tile([1, DG], BF, tag="vrow")
            fct = wp.tile([128, 2, D], BF, tag="fct")
            fcb = wp.tile([1, D], BF, tag="fcb")
            bq = wp.tile([128, 2, 1], F32, tag="bq")
            bk = wp.tile([128, 2, 1], F32, tag="bk")
            nc.sync.dma_start(out=wqt[:], in_=WQT.rearrange("(kt p) m -> p kt m", p=128))
            nc.sync.dma_start(out=bq[:], in_=BQ.rearrange("(i p) o -> p i o", p=128))

            def load_half(src, half, name):
                # one DMA per k-tile so the 8-matmul accumulation chain can
                # start as soon as its first operand lands
                t = xp.tile([128, 8, 1024], BF, tag="xt", name=name)
                for kt in range(8):
                    nc.sync.dma_start(
                        out=t[:, kt],
                        in_=src[kt * 128:(kt + 1) * 128,
                                half * 1024:(half + 1) * 1024],
                    )
                return t

            def proj_qk_half(xt, half, wt, bias, dst):
                for mch in range(2):
                    for sl in range(2):
                        s = half * 2 + sl
                        p = pout.tile([128, 512], F32, tag="out", name="pqk")
                        for kt in range(8):
                            nc.tensor.matmul(
                                p[:],
                                wt[:, kt, mch * 128:(mch + 1) * 128],
                                xt[:, kt, sl * 512:(sl + 1) * 512],
                                start=(kt == 0),
                                stop=(kt == 7),
                            )
                        nc.vector.tensor_scalar_add(
                            dst[mch][:, s * 512:(s + 1) * 512], p[:], bias[:, mch]
                        )

            t = load_half(XTQ, 0, "xtq")
            nc.sync.dma_start(out=wkt[:], in_=WKT.rearrange("(kt p) m -> p kt m", p=128))
            nc.sync.dma_start(out=bk[:], in_=BK.rearrange("(i p) o -> p i o", p=128))
            proj_qk_half(t, 0, wqt, bq, qt_sb)
            t = load_half(XTQ, 1, "xtq")
            nc.sync.dma_start(out=wvt[:], in_=WVT[0:D].rearrange("(kt p) m -> p kt m", p=128))
            nc.sync.dma_start(out=vrow[:], in_=WVT[D:D + 1])
            proj_qk_half(t, 1, wqt, bq, qt_sb)
            for half in range(2):
                t = load_half(XTK, half, "xtk")
                proj_qk_half(t, half, wkt, bk, kt_sb)

            # ---------------- v projection ----------------
            xr = xp.tile([1, L], BF, tag="xtr", bufs=1, name="xr")
            nc.sync.dma_start(out=xr[:], in_=XTV[D:D + 1])
            nc.sync.dma_start(out=fct[:], in_=FCT[0:DG].rearrange("(i p) m -> p i m", p=128))
            nc.sync.dma_start(out=fcb[:], in_=FCT[DG:DG + 1])

            for half in range(2):
                xtv = load_half(XTV, half, "xtv")
                for lbl in range(8):
                    lb = half * 8 + lbl
                    p = pout.tile([128, DG], F32, tag="out", name="pv")
                    for kt in range(8):
                        nc.tensor.matmul(
                            p[:],
                            xtv[:, kt, lbl * 128:(lbl + 1) * 128],
                            wvt[:, kt],
                            start=(kt == 0),
                            stop=(zv and kt == 7),
                        )
                    if not zv:
                        nc.tensor.matmul(
                            p[:], xr[:, lb * 128:(lb + 1) * 128], vrow[:],
                            start=False, stop=True,
                        )
                    nc.vector.tensor_copy(
                        v_sb[:, lb, :, 0:64], p[:].rearrange("p (h d) -> p h d", h=GH)
                    )

            # 0/1 chunks for mixed mask blocks, one DMA + one convert
            m01_idx = {qk: i for i, qk in enumerate(mixed)}
            m01_all = wp.tile([128, nmix, 128], BF, tag="m01")
            if mixed:
                mstage = wp.tile([128, nmix, 128], U8, tag="mstage")
                nc.sync.dma_start(out=mstage[:], in_=MCHUNKS.rearrange("n p c -> p n c"))
                nc.scalar.copy(m01_all[:], mstage[:])

            # ---------------- attention ----------------
            def first_col(kb, s):
                if kb == span_kbs[s][0]:
                    return 0
                j0 = next(j for j in range(4) if cls[4 * s + j, kb])
                return j0 * 128

            def normalize(hi, ho, s, outp):
                srow = smp.tile([1, 512], F32, tag="srow", name="srow")
                nc.vector.tensor_copy(srow[:], outp[64:65, :])
                rc32 = smp.tile([1, 512], F32, tag="rc32", name="rc32")
                nc.vector.reciprocal_approx_fast(rc32[:], srow[:])
                rcb = smp.tile([1, 512], BF, tag="rcb", name="rcb")
                nc.vector.tensor_copy(rcb[:], rc32[:])
                bcp = psc.tile([64, 512], F32, tag="sc", name="bcp")
                nc.tensor.matmul(bcp[:], ones64[:], rcb[:], start=True, stop=True)
                bcs = evp.tile([64, 512], BF, tag="bcs", name="bcs")
                nc.vector.tensor_copy(bcs[:], bcp[:])
                nc.vector.tensor_mul(
                    ctx_sb[hi][ho:ho + 64, s * 512:(s + 1) * 512],
                    outp[0:64, :],
                    bcs[:],
                )

            pending = None
            for h in range(GH):
                hi, ho = h // 2, (h % 2) * 64
                for s in range(NSPAN):
                    kbs = span_kbs[s]
                    outp = pout.tile([65, 512], F32, tag="out", name=f"out{h}_{s}")
                    ngrp = (len(kbs) + 1) // 2
                    for gi in range(ngrp):
                        grp = kbs[gi * 2:gi * 2 + 2]
                        scp = psc.tile([128, 1024], F32, tag="sc", name="scp")
                        pt = ptp.tile([128, 1024], BF, tag="pt", name="pt")
                        c0s = [first_col(kb, s) for kb in grp]
                        for idx, kb in enumerate(grp):
                            off = idx * 512
                            nc.tensor.matmul(
                                scp[:, off + c0s[idx]:off + 512],
                                kt_sb[hi][ho:ho + 64, kb * 128:(kb + 1) * 128],
                                qt_sb[hi][ho:ho + 64, s * 512 + c0s[idx]:(s + 1) * 512],
                                start=True,
                                stop=True,
                            )
                        lo = c0s[0]
                        end = (len(grp) - 1) * 512 + 512
                        nc.scalar.activation(pt[:, lo:end], scp[:, lo:end], Exp, scale=0.125)
                        for idx, kb in enumerate(grp):
                            off = idx * 512
                            for j in range(c0s[idx] // 128, 4):
                                qb = 4 * s + j
                                c = cls[qb, kb]
                                if c == 0:
                                    nc.vector.memset(pt[:, off + j * 128:off + (j + 1) * 128], 0.0)
                                elif c == 2:
                                    nc.vector.tensor_mul(
                                        pt[:, off + j * 128:off + (j + 1) * 128],
                                        pt[:, off + j * 128:off + (j + 1) * 128],
                                        m01_all[:, m01_idx[(qb, kb)], :],
                                    )
                            nc.tensor.matmul(
                                outp[:, c0s[idx]:],
                                v_sb[:, kb, h, :],
                                pt[:, off + c0s[idx]:off + 512],
                                start=(kb == kbs[0]),
                                stop=(kb == kbs[-1]),
                                skip_group_check=True,
                            )
                        # flush the previous span's normalize mid-stream so its
                        # reciprocal chain never stalls the PE
                        if gi == min(1, ngrp - 1) and pending is not None:
                            normalize(*pending)
                            pending = None
                    pending = (hi, ho, s, outp)
            if pending is not None:
                normalize(*pending)

            # ---------------- fc ----------------
            for lb in range(NB):
                ys = evp.tile([128, 1024], F32, tag="ys")
                for nh in range(2):
                    yp = pout.tile([128, 512], F32, tag="out", name="yp")
                    nc.tensor.matmul(
                        yp[:], ctx_sb[0][:, lb * 128:(lb + 1) * 128],
                        fct[:, 0, nh * 512:(nh + 1) * 512], start=True, stop=False,
                    )
                    nc.tensor.matmul(
                        yp[:], ctx_sb[1][:, lb * 128:(lb + 1) * 128],
                        fct[:, 1, nh * 512:(nh + 1) * 512], start=False, stop=zf,
                    )
                    if not zf:
                        nc.tensor.matmul(
                            yp[:], ctx1[:, lb * 128:(lb + 1) * 128],
                            fcb[:, nh * 512:(nh + 1) * 512], start=False, stop=True,
                        )
                    nc.vector.tensor_copy(ys[:, nh * 512:(nh + 1) * 512], yp[:])
                nc.sync.dma_start(out=Y[lb * 128:(lb + 1) * 128, :], in_=ys[:])

    nc.compile()
    return nc


def kernel(Q, K, V, mask, Wq_w, Wq_b, Wk_w, Wk_b, Wv_w, Wv_b, fc_w, fc_b):
    global LAST_EXEC_NS
    Q = np.asarray(Q, np.float32)
    K = np.asarray(K, np.float32)
    V = np.asarray(V, np.float32)
    mask2d = np.asarray(mask).reshape(L, L).astype(bool)
    Wq_w = np.asarray(Wq_w, np.float32)
    Wq_b = np.asarray(Wq_b, np.float32)
    Wk_w = np.asarray(Wk_w, np.float32)
    Wk_b = np.asarray(Wk_b, np.float32)
    Wv_w = np.asarray(Wv_w, np.float32)
    Wv_b = np.asarray(Wv_b, np.float32)
    fc_w = np.asarray(fc_w, np.float32)
    fc_b = np.asarray(fc_b, np.float32)

    cls = _classify(mask2d)
    zv = not Wv_b.any()
    zf = not fc_b.any()
    key = (cls.tobytes(), zv, zf)
    if key not in _CACHE:
        _CACHE[key] = _build(cls, zv, zf)
    nc = _CACHE[key]

    bf = ml_dtypes.bfloat16
    mixed = _mixed_list(cls)
    if mixed:
        mchunks = np.stack([
            np.ascontiguousarray(mask2d[qb * 128:(qb + 1) * 128, kb * 128:(kb + 1) * 128].T)
            for qb, kb in mixed
        ]).astype(np.uint8)
    else:
        mchunks = np.zeros((1, 128, 128), np.uint8)
    ones_row = np.ones((1, L), np.float32)

    xt = {}
    for b in range(2):
        xt[("Q", b)] = np.ascontiguousarray(Q[b].T).astype(bf)
        xt[("K", b)] = np.ascontiguousarray(K[b].T).astype(bf)
        xt[("V", b)] = np.concatenate([np.ascontiguousarray(V[b].T), ones_row], 0).astype(bf)

    in_maps = []
    for c in range(8):
        b, g = c // 4, c % 4
        sl = slice(g * DG, (g + 1) * DG)
        fc_last = fc_b[None, :] if g == 0 else np.zeros((1, D), np.float32)
        in_maps.append({
            "XTQ": xt[("Q", b)],
            "XTK": xt[("K", b)],
            "XTV": xt[("V", b)],
            "WQT": np.ascontiguousarray(Wq_w[sl, :].T).astype(bf),
            "WKT": np.ascontiguousarray(Wk_w[sl, :].T).astype(bf),
            "WVT": np.concatenate(
                [np.ascontiguousarray(Wv_w[sl, :].T), Wv_b[sl][None, :]], 0
            ).astype(bf),
            "BQ": np.ascontiguousarray(Wq_b[sl].reshape(DG, 1)),
            "BK": np.ascontiguousarray(Wk_b[sl].reshape(DG, 1)),
            "FCT": np.concatenate(
                [np.ascontiguousarray(fc_w[:, sl].T), fc_last], 0
            ).astype(bf),
            "MCHUNKS": mchunks,
        })

    if TRACE:
        _install_ntff_hook()
    res = bass_utils.run_bass_kernel_spmd(
        nc, in_maps, core_ids=list(range(8)),
        trace=TRACE, trace_cores=list(range(8)) if TRACE else None,
    )
    LAST_EXEC_NS = res.exec_time_ns

    out = np.zeros((2, L, D), np.float32)
    for c in range(8):
        out[c // 4] += res.results[c]["Y"]
    return out
